# revision 1
# baseline (speedup 1.0000x reference)
"""BiLSTM Enc-Dec + CRF NLL loss on 2 Trainium2 cores (SPMD, fwd/bwd split).

Strategy
--------
Batch=1 sequence, T=2048. The four BiLSTM scans (enc L0 -> enc L1 -> dec L0
-> dec L1) are inherently sequential in time; within each layer the forward
and backward direction are independent. So: core 0 runs all forward-direction
scans, core 1 runs all backward-direction scans, with one identical (symmetric)
SPMD program. Direction asymmetry is absorbed into per-core *data*:
  - core 1 receives the embedding sequence time-reversed, so its "forward"
    scan IS the backward scan;
  - per-core weight tensors are the own-direction slices, with gate rows
    permuted to [i, f, o, g] so sigmoid covers one contiguous slab;
  - cross-core exchanges (layer outputs, final states, feats partials) use
    AllGather / AllReduce on internal DRAM bounce buffers.
Input projections x @ W_ih^T for a whole layer are big parallel matmuls
computed once per stage into DRAM (fp32), streamed into SBUF in windows
during the scan. The recurrent matvec h @ W_hh^T runs on the tensor engine
as 64 [128x128] bf16 weight-stationary matmuls per step, unrolled U steps
per hardware-loop iteration to amortize the loop barrier.

The CRF forward pass runs in the linear domain: alpha' = exp(trans) @ alpha
(a single stationary 48x48 matmul per step) times exp(feats_t), renormalized
each step by its sum (one more tiny matmul with a ones vector); log of the
normalizer is accumulated on the host in float64. The CRF score term (tag
path score) is computed on the host from the device-computed feats.
"""

import sys

sys.path.insert(0, "/opt/trn_rl_repo")

import numpy as np
import ml_dtypes

import concourse.bacc as bacc
import concourse.mybir as mybir
from concourse.bass import ds
from concourse.tile import TileContext
from concourse.bass_utils import run_bass_kernel_spmd

# problem dims (hardcoded per spec)
T = 2048
ELMO = 1024
H = 512
POS = 64
K = 48
S = 50
L = 2
NEG = -10000.0
START_IDX, END_IDX = 0, 1

Din0 = ELMO + POS  # 1088
K0C = 9  # ceil(1088/128) k-tiles for layer-0 input (padded to 1152)
HC = 4  # h chunks of 128
G = 4 * H  # 2048 gates
GC = 16  # gate chunks of 128
U = 8  # scan steps unrolled per hardware-loop iteration
CH = 128  # scan steps per xp SBUF window
UCRF = 16

bf16 = mybir.dt.bfloat16
f32 = mybir.dt.float32
AF = mybir.ActivationFunctionType
ALU = mybir.AluOpType

_CACHE = {}


# ----------------------------------------------------------------------------
# host-side weight preparation
# ----------------------------------------------------------------------------

def _perm_gates(a):
    """reorder gate rows [i,f,g,o] -> [i,f,o,g] along axis 0 (size 4H)."""
    return np.concatenate([a[0:H], a[H : 2 * H], a[3 * H : 4 * H], a[2 * H : 3 * H]], 0)


def _tile_kT(wT, nk):
    """[Ktot, M] -> [128, nk*M] with col kc*M + m = wT[kc*128 + p, m]."""
    Ktot, M = wT.shape
    assert Ktot == nk * 128
    return np.ascontiguousarray(wT.reshape(nk, 128, M).transpose(1, 0, 2).reshape(128, nk * M))


def _prep_core(inputs, d):
    """Build the per-core input map for direction d (0=fwd core, 1=bwd core)."""
    f = np.float32
    ins = {}
    sentence = inputs["sentence"].astype(f)
    pos_emb = inputs["pos_emb"].astype(f)
    speech = inputs["speech_tags"].astype(np.int64)
    embeds = np.concatenate([sentence, pos_emb[speech]], axis=1)  # (T, 1088)
    if d == 1:
        embeds = embeds[::-1]
    embT = np.zeros((K0C * 128, T), f)
    embT[:Din0] = embeds.T
    ins["embT"] = _tile_kT(embT, K0C).astype(ml_dtypes.bfloat16)

    for model in ("enc", "dec"):
        for layer in (0, 1):
            whh = _perm_gates(inputs[f"{model}_w_hh{layer}"][d].astype(f))  # (2048, 512)
            ins[f"whhT_{model}{layer}"] = _tile_kT(
                np.ascontiguousarray(whh.T), HC
            ).astype(ml_dtypes.bfloat16)
            b = _perm_gates(
                (inputs[f"{model}_b_ih{layer}"][d] + inputs[f"{model}_b_hh{layer}"][d]).astype(f)
            )
            ins[f"bias_{model}{layer}"] = np.ascontiguousarray(
                b.reshape(GC, 128).T
            ).astype(f)  # [128,16] col mc
        wih0 = _perm_gates(inputs[f"{model}_w_ih0"][d].astype(f))  # (2048, 1088)
        w0T = np.zeros((K0C * 128, G), f)
        w0T[:Din0] = wih0.T
        ins[f"wih0T_{model}"] = _tile_kT(w0T, K0C).astype(ml_dtypes.bfloat16)
        wih1 = _perm_gates(inputs[f"{model}_w_ih1"][d].astype(f))  # (2048, 1024)
        own = wih1[:, d * H : (d + 1) * H]
        peer = wih1[:, (1 - d) * H : (2 - d) * H]
        ins[f"wih1T_own_{model}"] = _tile_kT(np.ascontiguousarray(own.T), HC).astype(
            ml_dtypes.bfloat16
        )
        ins[f"wih1T_peer_{model}"] = _tile_kT(np.ascontiguousarray(peer.T), HC).astype(
            ml_dtypes.bfloat16
        )

    # e2h/e2c: rows = own dec init states, cols permuted to AllGather order.
    # AG order of the 2048-dim enc state: [c0_l0, c0_l1, c1_l0, c1_l1]
    # (c0 = fwd dir, c1 = bwd dir); PyTorch flat order is [l0f, l0b, l1f, l1b].
    col_perm = np.concatenate(
        [
            np.arange(0, H),  # l0f
            np.arange(2 * H, 3 * H),  # l1f
            np.arange(H, 2 * H),  # l0b
            np.arange(3 * H, 4 * H),  # l1b
        ]
    )
    # own dec-init rows: init_h.reshape(2L, H)[j] is state for scan order
    # [dl0_f, dl0_b, dl1_f, dl1_b]; core d needs rows for [dl0 dir d, dl1 dir d]
    row_sel = np.concatenate([np.arange(d * H, (d + 1) * H), np.arange((2 + d) * H, (3 + d) * H)])
    for nm in ("e2h", "e2c"):
        w = inputs[f"{nm}_w"].astype(f)[row_sel][:, col_perm]  # (1024, 2048)
        ins[f"{nm}T"] = _tile_kT(np.ascontiguousarray(w.T), GC).astype(ml_dtypes.bfloat16)
        b = inputs[f"{nm}_b"].astype(f)[row_sel]  # (1024,)
        ins[f"{nm}_b"] = np.ascontiguousarray(b.reshape(8, 128).T).astype(f)  # [128, 8]

    # feats weights: rank0 half multiplies fwd-core outputs, rank1 half the
    # bwd-core outputs (identical on both cores; feats computed redundantly)
    h2t = inputs["h2t_w"].astype(f)
    ins["h2tT_r0"] = _tile_kT(np.ascontiguousarray(h2t[:, 0:H].T), HC).astype(ml_dtypes.bfloat16)
    ins["h2tT_r1"] = _tile_kT(np.ascontiguousarray(h2t[:, H:].T), HC).astype(ml_dtypes.bfloat16)
    ins["h2t_b"] = inputs["h2t_b"].astype(f).reshape(K, 1)

    trans = inputs["transitions"].astype(f)
    ins["transT"] = np.ascontiguousarray(trans.T)
    ins["transEnd"] = np.ascontiguousarray(trans[END_IDX].reshape(K, 1))
    a0 = np.full((K, 1), 0.0, f)
    a0[:, 0] = 0.0
    a0[START_IDX, 0] = 1.0
    ins["alpha0"] = a0
    return ins


# ----------------------------------------------------------------------------
# device program
# ----------------------------------------------------------------------------

def build():
    nc = bacc.Bacc("TRN2", target_bir_lowering=False, num_devices=2)

    def din(name, shape, dt=bf16):
        return nc.dram_tensor(name, shape, dt, kind="ExternalInput")

    embT_d = din("embT", [128, K0C * T])
    whh_d = {s: din(f"whhT_{s}", [128, HC * G]) for s in ("enc0", "enc1", "dec0", "dec1")}
    bias_d = {s: din(f"bias_{s}", [128, GC], f32) for s in ("enc0", "enc1", "dec0", "dec1")}
    wih0_d = {m: din(f"wih0T_{m}", [128, K0C * G]) for m in ("enc", "dec")}
    wih1o_d = {m: din(f"wih1T_own_{m}", [128, HC * G]) for m in ("enc", "dec")}
    wih1p_d = {m: din(f"wih1T_peer_{m}", [128, HC * G]) for m in ("enc", "dec")}
    e2hT_d = din("e2hT", [128, GC * 1024])
    e2cT_d = din("e2cT", [128, GC * 1024])
    e2hb_d = din("e2h_b", [128, 8], f32)
    e2cb_d = din("e2c_b", [128, 8], f32)
    h2tT_r0_d = din("h2tT_r0", [128, HC * K])
    h2tT_r1_d = din("h2tT_r1", [128, HC * K])
    h2tb_d = din("h2t_b", [K, 1], f32)
    transT_d = din("transT", [K, K], f32)
    transEnd_d = din("transEnd", [K, 1], f32)
    alpha0_d = din("alpha0", [K, 1], f32)

    feats_out = nc.dram_tensor("feats", [K, T], f32, kind="ExternalOutput")
    lnS_out = nc.dram_tensor("lnS", [1, T], f32, kind="ExternalOutput")
    zfin_out = nc.dram_tensor("zfin", [1, 1], f32, kind="ExternalOutput")

    # internal DRAM
    xp_a = nc.dram_tensor("xp_a", [128, GC * T], f32)  # enc0 / enc1 / dec1
    xp_b = nc.dram_tensor("xp_b", [128, GC * T], f32)  # dec0
    hs_ag_in = nc.dram_tensor("hs_ag_in", [128, HC * (T + 1)], bf16)
    hs_ag_out = nc.dram_tensor("hs_ag_out", [256, HC * (T + 1)], bf16)
    fin_ag_in = nc.dram_tensor("fin_ag_in", [128, 16], f32)
    fin_ag_out = nc.dram_tensor("fin_ag_out", [256, 16], f32)

    RG = [[0, 1]]

    with TileContext(nc) as tc:
        with (
            tc.tile_pool(name="pw", bufs=1) as pw,  # persistent weights/state
            tc.tile_pool(name="slab", bufs=1) as slab_pool,  # wih0 scratch 4.5MB
            tc.tile_pool(name="slab1", bufs=1) as slab1_pool,  # wih1 own
            tc.tile_pool(name="slab2", bufs=1) as slab2_pool,  # wih1 peer / e2h
            tc.tile_pool(name="hs", bufs=2) as hs_pool,
            tc.tile_pool(name="peer", bufs=1) as peer_pool,
            tc.tile_pool(name="xpw", bufs=2) as xpw_pool,
            tc.tile_pool(name="psx", bufs=2, space="PSUM") as psx_pool,  # xp matmuls
            tc.tile_pool(name="pss", bufs=4, space="PSUM") as pss_pool,  # scan
            tc.tile_pool(name="psm", bufs=2, space="PSUM") as psm_pool,  # crf
        ):
            # ---- persistent loads (whh loaded on demand, one shared slot)
            bias = {}
            for s in ("enc0", "enc1", "dec0", "dec1"):
                bias[s] = pw.tile([128, GC], f32, name=f"bias_{s}")
                nc.sync.dma_start(out=bias[s], in_=bias_d[s][:, :])

            # ---- xp matmul helper: out_dram[:, mc*T + t] over given k-slabs
            def xp_matmul(out_dram, slabs, bias_tile):
                """slabs: list of (sbuf_slab, nk, rhs_fn) triples contracting
                consecutive k-ranges; rhs_fn(kc, t0, n) -> AP [128, n] moving."""
                NT = 512
                for tb in range(T // NT):
                    t0 = tb * NT
                    for mc in range(GC):
                        ps = psx_pool.tile([128, NT], f32, tag="psx", name=f"psx_{tb}_{mc}")
                        first = True
                        for slab, nk, rhs_fn in slabs:
                            for kc in range(nk):
                                nc.tensor.matmul(
                                    ps,
                                    slab[:, kc * G + mc * 128 : kc * G + (mc + 1) * 128],
                                    rhs_fn(kc, t0, NT),
                                    start=first,
                                    stop=(slab is slabs[-1][0]) and kc == nk - 1,
                                )
                                first = False
                        st = xpw_pool.tile([128, NT], f32, tag="xstage", name=f"xst_{tb}_{mc}")
                        nc.vector.tensor_scalar(
                            out=st, in0=ps, scalar1=bias_tile[:, mc : mc + 1],
                            scalar2=None, op0=ALU.add,
                        )
                        nc.sync.dma_start(
                            out=out_dram[:, mc * T + t0 : mc * T + t0 + NT], in_=st
                        )

            # ---- P0: layer-0 xp for enc and dec (embT and wih0 streamed
            # in windows; weight window per (tb, mc): [128, K0C, 128])
            embr = embT_d[:, :].rearrange("p (k t) -> p k t", k=K0C)
            NT = 512
            for model, out_dram in (("enc", xp_a), ("dec", xp_b)):
                w0r = wih0_d[model][:, :].rearrange("p (k m) -> p k m", k=K0C)
                for tb in range(T // NT):
                    t0 = tb * NT
                    ew = xpw_pool.tile([128, K0C, NT], bf16, tag="win", name=f"ew_{model}_{tb}")
                    nc.sync.dma_start(out=ew, in_=embr[:, :, t0 : t0 + NT])
                    for mc in range(GC):
                        ww = xpw_pool.tile(
                            [128, K0C, 128], bf16, tag="wwin", name=f"ww_{model}_{tb}_{mc}"
                        )
                        nc.sync.dma_start(
                            out=ww, in_=w0r[:, :, mc * 128 : (mc + 1) * 128]
                        )
                        ps = psx_pool.tile([128, NT], f32, tag="psx", name=f"psx0_{model}_{tb}_{mc}")
                        for kc in range(K0C):
                            nc.tensor.matmul(
                                ps, ww[:, kc, :], ew[:, kc, :],
                                start=(kc == 0), stop=(kc == K0C - 1),
                            )
                        st = xpw_pool.tile([128, NT], f32, tag="xstage", name=f"x0_{model}_{tb}_{mc}")
                        nc.vector.tensor_scalar(
                            out=st, in0=ps, scalar1=bias[f"{model}0"][:, mc : mc + 1],
                            scalar2=None, op0=ALU.add,
                        )
                        nc.sync.dma_start(
                            out=out_dram[:, mc * T + t0 : mc * T + t0 + NT], in_=st
                        )

            # ---- scan helper
            def scan(s, xp_dram, Hs, c, h0_src=None, c0_src=None):
                """Run one LSTM direction scan. Hs: [128, HC*(T+1)] bf16 tile;
                c: [128, HC] f32 tile. h0/c0 default zero."""
                W = slab1_pool.tile([128, HC * G], bf16, tag="whh", name=f"whh_{s}")
                nc.sync.dma_start(out=W, in_=whh_d[s][:, :])
                if h0_src is None:
                    nc.vector.memset(Hs[:, 0:HC], 0.0)
                    nc.vector.memset(c, 0.0)
                else:
                    nc.vector.tensor_copy(Hs[:, 0:HC], h0_src)
                    nc.vector.tensor_copy(c, c0_src)
                gsb = pw.tile([128, GC], f32, tag="gsb", name=f"gsb_{s}")
                sig = pw.tile([128, 12], f32, tag="sig", name=f"sig_{s}")
                tng = pw.tile([128, 4], f32, tag="tng", name=f"tng_{s}")
                tt1 = pw.tile([128, 4], f32, tag="tt1", name=f"tt1_{s}")
                tt2 = pw.tile([128, 4], f32, tag="tt2", name=f"tt2_{s}")
                tnc = pw.tile([128, 4], f32, tag="tnc", name=f"tnc_{s}")
                for w in range(T // CH):
                    t0 = w * CH
                    xw = xpw_pool.tile([128, GC, CH], f32, tag="win", name=f"xw_{s}_{w}")
                    nc.sync.dma_start(
                        out=xw,
                        in_=xp_dram[:, :].rearrange("p (g t) -> p g t", g=GC)[
                            :, :, t0 : t0 + CH
                        ],
                    )
                    with tc.For_i(0, CH // U) as iv:
                        for u in range(U):
                            ps = pss_pool.tile([128, GC], f32, tag="ps", name=f"ps_{s}_{u}")
                            # col of h_{t-1}: HC*(t0 + iv*U + u) + kc
                            hbase = HC * t0 + HC * U * iv + HC * u
                            for mc in range(GC):
                                for kc in range(HC):
                                    nc.tensor.matmul(
                                        ps[:, mc : mc + 1],
                                        W[:, kc * G + mc * 128 : kc * G + (mc + 1) * 128],
                                        Hs[:, ds(hbase + kc, 1)],
                                        start=(kc == 0),
                                        stop=(kc == HC - 1),
                                    )
                            nc.vector.tensor_tensor(
                                out=gsb, in0=ps, in1=xw[:, :, ds(U * iv + u, 1)], op=ALU.add
                            )
                            nc.scalar.activation(sig, gsb[:, 0:12], AF.Sigmoid)
                            nc.scalar.activation(tng, gsb[:, 12:16], AF.Tanh)
                            nc.vector.tensor_tensor(out=tt1, in0=sig[:, 4:8], in1=c, op=ALU.mult)
                            nc.vector.tensor_tensor(out=tt2, in0=sig[:, 0:4], in1=tng, op=ALU.mult)
                            nc.vector.tensor_tensor(out=c, in0=tt1, in1=tt2, op=ALU.add)
                            nc.scalar.activation(tnc, c, AF.Tanh)
                            nc.vector.tensor_tensor(
                                out=Hs[:, ds(hbase + HC * 1 + 0, 4)],
                                in0=sig[:, 8:12],
                                in1=tnc,
                                op=ALU.mult,
                            )

            # ---- AllGather of an Hs buffer; returns peer tile (peer's order).
            # Core-symmetric: peer block = (rank0 + rank1) - own, computed in
            # f32 chunks (exact for bf16 values).
            def exchange_hs(Hs, tagsuffix):
                nc.sync.dma_start(out=hs_ag_in[:, :], in_=Hs)
                nc.gpsimd.collective_compute(
                    "AllGather", ALU.bypass,
                    ins=[hs_ag_in[:, :]], outs=[hs_ag_out[:, :]], replica_groups=RG,
                )
                peer = peer_pool.tile(
                    [128, HC * (T + 1)], bf16, tag="peer", name=f"peer_{tagsuffix}"
                )
                CW = 1026  # 8 chunks cover HC*(T+1) = 8196 (last chunk 1014)
                for ci in range(8):
                    lo = ci * CW
                    hi = min(HC * (T + 1), lo + CW)
                    n = hi - lo
                    b0 = peer_pool.tile([128, CW], bf16, tag="pb0", name=f"pb0_{tagsuffix}_{ci}")
                    b1 = peer_pool.tile([128, CW], bf16, tag="pb1", name=f"pb1_{tagsuffix}_{ci}")
                    nc.sync.dma_start(out=b0[:, :n], in_=hs_ag_out[0:128, lo:hi])
                    nc.sync.dma_start(out=b1[:, :n], in_=hs_ag_out[128:256, lo:hi])
                    pf = peer_pool.tile([128, CW], f32, tag="pf", name=f"pf_{tagsuffix}_{ci}")
                    nc.vector.tensor_tensor(out=pf[:, :n], in0=b0[:, :n], in1=b1[:, :n], op=ALU.add)
                    nc.vector.tensor_tensor(out=pf[:, :n], in0=pf[:, :n], in1=Hs[:, lo:hi], op=ALU.subtract)
                    nc.vector.tensor_copy(peer[:, lo:hi], pf[:, :n])
                return peer

            # reversed-read AP into peer Hs outputs: own-time t in [t0, t0+n),
            # chunk kc -> peer col HC*(T - t) + kc, step -HC
            def peer_rev_ap(peer, kc, t0, n):
                return peer[:, :].rearrange("p (t c) -> p t c", c=HC)[
                    :, T - t0 : T - t0 - n : -1, kc
                ]

            # ---- ENC pipeline
            Hs_e0 = hs_pool.tile([128, HC * (T + 1)], bf16, tag="Hs", name="Hs_enc0")
            c_e0 = pw.tile([128, HC], f32, name="c_enc0")
            scan("enc0", xp_a, Hs_e0, c_e0)

            peer_e0 = exchange_hs(Hs_e0, "enc")
            own1 = slab1_pool.tile([128, HC * G], bf16, tag="slab1", name="w1o_enc")
            nc.sync.dma_start(out=own1, in_=wih1o_d["enc"][:, :])
            peer1 = slab2_pool.tile([128, HC * G], bf16, tag="slab2", name="w1p_enc")
            nc.sync.dma_start(out=peer1, in_=wih1p_d["enc"][:, :])
            xp_matmul(
                xp_a,
                [
                    (own1, HC, lambda kc, t0, n: Hs_e0[:, :].rearrange(
                        "p (t c) -> p t c", c=HC)[:, t0 + 1 : t0 + 1 + n, kc]),
                    (peer1, HC, lambda kc, t0, n: peer_rev_ap(peer_e0, kc, t0, n)),
                ],
                bias["enc1"],
            )
            Hs_e1 = hs_pool.tile([128, HC * (T + 1)], bf16, tag="Hs", name="Hs_enc1")
            c_e1 = pw.tile([128, HC], f32, name="c_enc1")
            scan("enc1", xp_a, Hs_e1, c_e1)

            # ---- finals AG + init-state matvecs
            fin = pw.tile([128, 16], f32, name="fin")
            nc.vector.tensor_copy(fin[:, 0:4], Hs_e0[:, HC * T : HC * T + 4])
            nc.vector.tensor_copy(fin[:, 4:8], Hs_e1[:, HC * T : HC * T + 4])
            nc.vector.tensor_copy(fin[:, 8:12], c_e0)
            nc.vector.tensor_copy(fin[:, 12:16], c_e1)
            nc.sync.dma_start(out=fin_ag_in[:, :], in_=fin)
            nc.gpsimd.collective_compute(
                "AllGather", ALU.bypass,
                ins=[fin_ag_in[:, :]], outs=[fin_ag_out[:, :]], replica_groups=RG,
            )
            enc_all = pw.tile([128, 32], f32, name="enc_all")
            nc.sync.dma_start(out=enc_all[:, 0:16], in_=fin_ag_out[0:128, :])
            nc.sync.dma_start(out=enc_all[:, 16:32], in_=fin_ag_out[128:256, :])

            e2hb = pw.tile([128, 8], f32, name="e2hb")
            nc.sync.dma_start(out=e2hb, in_=e2hb_d[:, :])
            e2cb = pw.tile([128, 8], f32, name="e2cb")
            nc.sync.dma_start(out=e2cb, in_=e2cb_d[:, :])
            # rhs columns in AG order: h cols = enc_all [0:8] + [16:24];
            # c cols = [8:16] + [24:32]. BUT enc_all must be bf16 for matmul.
            enc_all_bf = pw.tile([128, 32], bf16, name="enc_all_bf")
            nc.vector.tensor_copy(enc_all_bf, enc_all)
            hcols = list(range(0, 8)) + list(range(16, 24))
            ccols = list(range(8, 16)) + list(range(24, 32))
            init_h = pw.tile([128, 8], f32, name="init_h")
            init_c = pw.tile([128, 8], f32, name="init_c")
            for (wd, cols, bt, out_t) in (
                (e2hT_d, hcols, e2hb, init_h),
                (e2cT_d, ccols, e2cb, init_c),
            ):
                wr = wd[:, :].rearrange("p (k m) -> p k m", k=GC)
                ps = psx_pool.tile([128, 8], f32, tag="psx", name=f"ps_init_{out_t.name}")
                for m in range(8):
                    eww = xpw_pool.tile(
                        [128, GC, 128], bf16, tag="wwin", name=f"e2w_{out_t.name}_{m}"
                    )
                    nc.sync.dma_start(out=eww, in_=wr[:, :, m * 128 : (m + 1) * 128])
                    for kc in range(GC):
                        nc.tensor.matmul(
                            ps[:, m : m + 1],
                            eww[:, kc, :],
                            enc_all_bf[:, cols[kc] : cols[kc] + 1],
                            start=(kc == 0),
                            stop=(kc == GC - 1),
                        )
                nc.vector.tensor_tensor(out=out_t, in0=ps, in1=bt, op=ALU.add)
            init_h_bf = pw.tile([128, 8], bf16, name="init_h_bf")
            nc.vector.tensor_copy(init_h_bf, init_h)

            # ---- DEC pipeline
            Hs_d0 = hs_pool.tile([128, HC * (T + 1)], bf16, tag="Hs", name="Hs_dec0")
            c_d0 = pw.tile([128, HC], f32, name="c_dec0")
            scan("dec0", xp_b, Hs_d0, c_d0, init_h_bf[:, 0:4], init_c[:, 0:4])

            peer_d0 = exchange_hs(Hs_d0, "dec")
            own1d = slab1_pool.tile([128, HC * G], bf16, tag="slab1", name="w1o_dec")
            nc.sync.dma_start(out=own1d, in_=wih1o_d["dec"][:, :])
            peer1d = slab2_pool.tile([128, HC * G], bf16, tag="slab2", name="w1p_dec")
            nc.sync.dma_start(out=peer1d, in_=wih1p_d["dec"][:, :])
            xp_matmul(
                xp_a,
                [
                    (own1d, HC, lambda kc, t0, n: Hs_d0[:, :].rearrange(
                        "p (t c) -> p t c", c=HC)[:, t0 + 1 : t0 + 1 + n, kc]),
                    (peer1d, HC, lambda kc, t0, n: peer_rev_ap(peer_d0, kc, t0, n)),
                ],
                bias["dec1"],
            )
            Hs_d1 = hs_pool.tile([128, HC * (T + 1)], bf16, tag="Hs", name="Hs_dec1")
            c_d1 = pw.tile([128, HC], f32, name="c_dec1")
            scan("dec1", xp_a, Hs_d1, c_d1, init_h_bf[:, 4:8], init_c[:, 4:8])

            # ---- feats: AllGather dec-L1 outputs; each core computes the
            # full feats identically (rank0 block = fwd dir ascending, rank1
            # block = bwd dir, read time-reversed).
            nc.sync.dma_start(out=hs_ag_in[:, :], in_=Hs_d1)
            nc.gpsimd.collective_compute(
                "AllGather", ALU.bypass,
                ins=[hs_ag_in[:, :]], outs=[hs_ag_out[:, :]], replica_groups=RG,
            )
            r0b = peer_pool.tile([128, HC * (T + 1)], bf16, tag="peer", name="d1_r0")
            nc.sync.dma_start(out=r0b, in_=hs_ag_out[0:128, :])
            r1b = peer_pool.tile([128, HC * (T + 1)], bf16, tag="peerb", name="d1_r1")
            nc.sync.dma_start(out=r1b, in_=hs_ag_out[128:256, :])
            h2tT0 = pw.tile([128, HC * K], bf16, name="h2tT0")
            nc.sync.dma_start(out=h2tT0, in_=h2tT_r0_d[:, :])
            h2tT1 = pw.tile([128, HC * K], bf16, name="h2tT1")
            nc.sync.dma_start(out=h2tT1, in_=h2tT_r1_d[:, :])
            feats = pw.tile([K, T], f32, name="feats")
            NT = 512
            r0r = r0b[:, :].rearrange("p (t c) -> p t c", c=HC)
            r1r = r1b[:, :].rearrange("p (t c) -> p t c", c=HC)
            for tb in range(T // NT):
                t0 = tb * NT
                ps = psx_pool.tile([K, NT], f32, tag="psx", name=f"psf_{tb}")
                for kc in range(HC):
                    nc.tensor.matmul(
                        ps, h2tT0[:, kc * K : (kc + 1) * K],
                        r0r[:, t0 + 1 : t0 + 1 + NT, kc],
                        start=(kc == 0), stop=False,
                    )
                for kc in range(HC):
                    nc.tensor.matmul(
                        ps, h2tT1[:, kc * K : (kc + 1) * K],
                        r1r[:, T - t0 : T - t0 - NT : -1, kc],
                        start=False, stop=(kc == HC - 1),
                    )
                nc.vector.tensor_copy(feats[:, t0 : t0 + NT], ps)
            h2tb = pw.tile([K, 1], f32, name="h2tb")
            nc.sync.dma_start(out=h2tb, in_=h2tb_d[:, :])
            nc.vector.tensor_scalar(
                out=feats, in0=feats, scalar1=h2tb, scalar2=None, op0=ALU.add
            )
            nc.sync.dma_start(out=feats_out[:, :], in_=feats)

            # ---- CRF forward (linear domain)
            expF = pw.tile([K, T], f32, name="expF")
            nc.scalar.activation(expF, feats, AF.Exp)
            transT_sb = pw.tile([K, K], f32, name="transT_sb")
            nc.sync.dma_start(out=transT_sb, in_=transT_d[:, :])
            PexpT = pw.tile([K, K], f32, name="PexpT")
            nc.scalar.activation(PexpT, transT_sb, AF.Exp)
            transEnd_sb = pw.tile([K, 1], f32, name="transEnd_sb")
            nc.sync.dma_start(out=transEnd_sb, in_=transEnd_d[:, :])
            expTE = pw.tile([K, 1], f32, name="expTE")
            nc.scalar.activation(expTE, transEnd_sb, AF.Exp)
            alpha = pw.tile([K, 1], f32, name="alpha")
            nc.sync.dma_start(out=alpha, in_=alpha0_d[:, :])
            ones48 = pw.tile([K, K], f32, name="ones48")
            nc.vector.memset(ones48, 1.0)
            lnS_sb = pw.tile([1, T], f32, name="lnS_sb")
            ut = pw.tile([K, 1], f32, name="ut")
            rs = pw.tile([K, 1], f32, name="rs")

            with tc.For_i(0, T // UCRF) as iv:
                for u in range(UCRF):
                    psA = psm_pool.tile([K, 1], f32, tag="psA", name=f"psA_{u}")
                    nc.tensor.matmul(psA, PexpT, alpha, start=True, stop=True)
                    nc.vector.tensor_tensor(
                        out=ut, in0=psA, in1=expF[:, ds(UCRF * iv + u, 1)], op=ALU.mult
                    )
                    psS = psm_pool.tile([K, 1], f32, tag="psA", name=f"psS_{u}")
                    nc.tensor.matmul(psS, ones48, ut, start=True, stop=True)
                    nc.scalar.activation(lnS_sb[:, ds(UCRF * iv + u, 1)], psS[0:1, :], AF.Ln)
                    nc.vector.reciprocal(rs, psS)
                    nc.vector.tensor_tensor(out=alpha, in0=ut, in1=rs, op=ALU.mult)
            psZ = psm_pool.tile([1, 1], f32, tag="psA", name="psZ")
            nc.tensor.matmul(psZ, alpha, expTE, start=True, stop=True)
            zf = pw.tile([1, 1], f32, name="zf")
            nc.scalar.activation(zf, psZ, AF.Ln)
            nc.sync.dma_start(out=zfin_out[:, :], in_=zf)
            nc.sync.dma_start(out=lnS_out[:, :], in_=lnS_sb)
    nc.compile()
    return nc


# ----------------------------------------------------------------------------
# entry point
# ----------------------------------------------------------------------------

def _postprocess(r0, inputs):
    feats = r0["feats"].astype(np.float64)  # [K, T]
    lnS = r0["lnS"].astype(np.float64)[0]
    zfin = float(r0["zfin"][0, 0])
    Z = float(lnS.sum() + zfin)

    tags = np.asarray(inputs["tags"]).astype(np.int64)
    trans = np.asarray(inputs["transitions"]).astype(np.float64)
    ext = np.concatenate([[START_IDX], tags])
    score = trans[ext[1:], ext[:-1]].sum() + feats[tags, np.arange(T)].sum()
    score += trans[END_IDX, tags[-1]]
    return np.float32(Z - score)


def kernel(**inputs) -> np.ndarray:
    if "nc" not in _CACHE:
        _CACHE["nc"] = build()
    nc = _CACHE["nc"]
    in_maps = [_prep_core(inputs, 0), _prep_core(inputs, 1)]
    res = run_bass_kernel_spmd(nc, in_maps, [0, 1])
    return _postprocess(res.results[0], inputs)



# revision 5
# speedup vs baseline: 1.7531x; 1.7531x over previous
"""BiLSTM Enc-Dec + CRF NLL loss on ONE Trainium2 core (zero collectives).

Design (from microbenchmarking this hardware):
- Small matmuls with register-offset (loop-var) access patterns cost ~300ns
  each; with constant offsets ~15ns. So the recurrent h@W_hh matvec keeps h
  in a (U+1)-slot ping-pong buffer indexed by the Python-unrolled step index:
  all 64 PE matmuls per step use constant APs. The time-indexed history write
  (needed by the next layer's input projection) is a batched DVE copy per
  unrolled body (register offsets are cheap on DVE).
- Collectives cost ~41 ms fixed per execution on this stack, so everything
  runs on core 0: the fwd and bwd direction scans of each layer are
  interleaved step-by-step on one core, which hides most of each scan's
  serial chain latency in the other's engine gaps.
- The per-step xp (input projection) add is folded into PSUM via an identity
  matmul (start=True) so activations read gates straight from PSUM.
- CRF partition function: linear domain with renorm every 8 steps; split
  into a forward alpha recursion over t=0..1023 and a backward beta
  recursion over t=2047..1024, interleaved on the same engines; the host
  sums the logs of the stored norms in float64.
"""

import sys

sys.path.insert(0, "/opt/trn_rl_repo")

import numpy as np
import ml_dtypes

import concourse.bacc as bacc
import concourse.mybir as mybir
from concourse.bass import ds
from concourse.tile import TileContext
from concourse.bass_utils import run_bass_kernel_spmd

T = 2048
ELMO = 1024
H = 512
POS = 64
K = 48
S = 50
L = 2
NEG = -10000.0
START_IDX, END_IDX = 0, 1

Din0 = ELMO + POS  # 1088
K0C = 9  # ceil(1088/128)
HC = 4
G = 4 * H  # 2048
GC = 16
U = 8  # unrolled steps per hardware-loop body
CH = 128  # steps per xp window
NT = 512  # time-block for bulk matmuls
RN = 8  # CRF renorm cadence
TH = T // 2  # alpha/beta split point

bf16 = mybir.dt.bfloat16
f32 = mybir.dt.float32
AF = mybir.ActivationFunctionType
ALU = mybir.AluOpType

_CACHE = {}

STAGES = [("enc", 0), ("enc", 1), ("dec", 0), ("dec", 1)]


# ----------------------------------------------------------------------------
# host-side weight preparation
# ----------------------------------------------------------------------------

def _perm_gates(a):
    """reorder gate rows [i,f,g,o] -> [i,f,o,g] along axis 0 (size 4H)."""
    return np.concatenate([a[0:H], a[H : 2 * H], a[3 * H : 4 * H], a[2 * H : 3 * H]], 0)


def _tile_kT(wT, nk):
    """[Ktot, M] -> [128, nk*M] with col kc*M + m = wT[kc*128 + p, m]."""
    Ktot, M = wT.shape
    assert Ktot == nk * 128
    return np.ascontiguousarray(wT.reshape(nk, 128, M).transpose(1, 0, 2).reshape(128, nk * M))


def _prep(inputs):
    f = np.float32
    ins = {}
    sentence = inputs["sentence"].astype(f)
    pos_emb = inputs["pos_emb"].astype(f)
    speech = np.asarray(inputs["speech_tags"]).astype(np.int64)
    embeds = np.concatenate([sentence, pos_emb[speech]], axis=1)  # (T, 1088)
    embT = np.zeros((K0C * 128, T), f)
    embT[:Din0] = embeds.T
    ins["embT"] = _tile_kT(embT, K0C).astype(ml_dtypes.bfloat16)
    ins["ident"] = np.eye(128).astype(ml_dtypes.bfloat16)

    for m in ("enc", "dec"):
        for l in (0, 1):
            for d in (0, 1):
                whh = _perm_gates(inputs[f"{m}_w_hh{l}"][d].astype(f))
                ins[f"whhT_{m}{l}_{d}"] = _tile_kT(
                    np.ascontiguousarray(whh.T), HC
                ).astype(ml_dtypes.bfloat16)
                b = _perm_gates(
                    (inputs[f"{m}_b_ih{l}"][d] + inputs[f"{m}_b_hh{l}"][d]).astype(f)
                )
                ins[f"bias_{m}{l}_{d}"] = np.ascontiguousarray(
                    b.reshape(GC, 128).T
                ).astype(f)
        for d in (0, 1):
            wih0 = _perm_gates(inputs[f"{m}_w_ih0"][d].astype(f))  # (2048, 1088)
            w0T = np.zeros((K0C * 128, G), f)
            w0T[:Din0] = wih0.T
            ins[f"wih0T_{m}_{d}"] = _tile_kT(w0T, K0C).astype(ml_dtypes.bfloat16)
            wih1 = _perm_gates(inputs[f"{m}_w_ih1"][d].astype(f))  # (2048, 1024)
            wf = wih1[:, 0:H]  # multiplies fwd-dir L0 outputs
            wb = wih1[:, H : 2 * H]  # multiplies bwd-dir L0 outputs
            ins[f"wih1T_{m}_{d}_f"] = _tile_kT(
                np.ascontiguousarray(wf.T), HC
            ).astype(ml_dtypes.bfloat16)
            ins[f"wih1T_{m}_{d}_b"] = _tile_kT(
                np.ascontiguousarray(wb.T), HC
            ).astype(ml_dtypes.bfloat16)

    # e2h/e2c: natural order both sides. out rows = [dl0f dl0b dl1f dl1b],
    # in cols = [el0f el0b el1f el1b] (PyTorch flat order of (2L, H) states).
    for nm in ("e2h", "e2c"):
        w = inputs[f"{nm}_w"].astype(f)  # (2048, 2048)
        ins[f"{nm}T"] = _tile_kT(np.ascontiguousarray(w.T), GC).astype(ml_dtypes.bfloat16)
        ins[f"{nm}_b"] = np.ascontiguousarray(
            inputs[f"{nm}_b"].astype(f).reshape(GC, 128).T
        ).astype(f)

    h2t = inputs["h2t_w"].astype(f)  # (K, 1024)
    ins["h2tT_f"] = _tile_kT(np.ascontiguousarray(h2t[:, 0:H].T), HC).astype(
        ml_dtypes.bfloat16
    )
    ins["h2tT_b"] = _tile_kT(np.ascontiguousarray(h2t[:, H:].T), HC).astype(
        ml_dtypes.bfloat16
    )
    ins["h2t_b"] = inputs["h2t_b"].astype(f).reshape(K, 1)

    trans = inputs["transitions"].astype(np.float64)
    E = np.exp(trans).astype(f)  # E[next, prev]
    ins["EexpT"] = np.ascontiguousarray(E.T)  # lhsT for alpha: out = E @ x
    ins["Eexp"] = np.ascontiguousarray(E)  # lhsT for beta: out = E^T @ x
    ins["betaT"] = np.ascontiguousarray(E[END_IDX].reshape(K, 1))  # exp(trans[END])
    a0 = np.zeros((K, 1), f)
    a0[START_IDX, 0] = 1.0
    ins["alpha0"] = a0
    ins["ones48"] = np.ones((K, K), f)
    return ins


# ----------------------------------------------------------------------------
# device program
# ----------------------------------------------------------------------------

def build():
    import os
    skips = set(os.environ.get("BK_SKIP", "").split(","))
    nc = bacc.Bacc("TRN2", target_bir_lowering=False, num_devices=1)

    def din(name, shape, dt=bf16):
        return nc.dram_tensor(name, shape, dt, kind="ExternalInput")

    embT_d = din("embT", [128, K0C * T])
    ident_d = din("ident", [128, 128])
    whh_d = {
        (m, l, d): din(f"whhT_{m}{l}_{d}", [128, HC * G])
        for m in ("enc", "dec") for l in (0, 1) for d in (0, 1)
    }
    bias_d = {
        (m, l, d): din(f"bias_{m}{l}_{d}", [128, GC], f32)
        for m in ("enc", "dec") for l in (0, 1) for d in (0, 1)
    }
    wih0_d = {(m, d): din(f"wih0T_{m}_{d}", [128, K0C * G]) for m in ("enc", "dec") for d in (0, 1)}
    wih1_d = {
        (m, d, s): din(f"wih1T_{m}_{d}_{s}", [128, HC * G])
        for m in ("enc", "dec") for d in (0, 1) for s in ("f", "b")
    }
    e2hT_d = din("e2hT", [128, GC * G])
    e2cT_d = din("e2cT", [128, GC * G])
    e2hb_d = din("e2h_b", [128, GC], f32)
    e2cb_d = din("e2c_b", [128, GC], f32)
    h2tT_f_d = din("h2tT_f", [128, HC * K])
    h2tT_b_d = din("h2tT_b", [128, HC * K])
    h2tb_d = din("h2t_b", [K, 1], f32)
    EexpT_d = din("EexpT", [K, K], f32)
    Eexp_d = din("Eexp", [K, K], f32)
    betaT_d = din("betaT", [K, 1], f32)
    alpha0_d = din("alpha0", [K, 1], f32)
    ones48_d = din("ones48", [K, K], f32)

    feats_out = nc.dram_tensor("feats", [K, T], f32, kind="ExternalOutput")
    NSA = TH // RN  # 128 alpha norms
    NSB = (T - TH) // RN  # 128 beta norms
    snorm_out = nc.dram_tensor("snorm", [1, NSA + NSB + 1], f32, kind="ExternalOutput")

    # internal DRAM xp buffers: one per (stage, dir)
    xp_dram = {}
    for m in ("enc", "dec"):
        for l in (0, 1):
            for d in (0, 1):
                xp_dram[(m, l, d)] = nc.dram_tensor(f"xp_{m}{l}_{d}", [128, GC * T], bf16)

    with TileContext(nc) as tc:
        with (
            tc.tile_pool(name="pw", bufs=1) as pw,
            tc.tile_pool(name="wslab", bufs=2) as wslab_pool,  # whh / wih1 slabs
            tc.tile_pool(name="hs", bufs=3) as hs_pool,
            tc.tile_pool(name="win", bufs=2) as win_pool,  # streamed emb windows
            tc.tile_pool(name="w0s", bufs=2) as w0s_pool,  # wih0 half-slab windows
            tc.tile_pool(name="xw", bufs=2) as xw_pool,  # xp scan windows
            tc.tile_pool(name="xst", bufs=6) as xst_pool,  # xp store staging
            tc.tile_pool(name="psx", bufs=2, space="PSUM") as psx_pool,
            tc.tile_pool(name="pss", bufs=3, space="PSUM") as pss_pool,
        ):
            ident = pw.tile([128, 128], bf16, name="ident")
            nc.sync.dma_start(out=ident, in_=ident_d[:, :])

            # ================= P0: layer-0 xp for all 4 (model, dir) =========
            embr = embT_d[:, :].rearrange("p (k t) -> p k t", k=K0C)
            for m in ("enc", "dec") if "p0" not in skips else ():
                for d in (0, 1):
                    bias = pw.tile([128, GC], f32, tag="bias0", name=f"b0_{m}{d}")
                    nc.sync.dma_start(out=bias, in_=bias_d[(m, 0, d)][:, :])
                    w0r = wih0_d[(m, d)][:, :].rearrange("p (k g) -> p k g", k=K0C)
                    w0h = []
                    for half in (0, 1):
                        w0t = w0s_pool.tile(
                            [128, K0C, G // 2], bf16, tag="w0h", name=f"w0_{m}{d}_{half}"
                        )
                        nc.sync.dma_start(
                            out=w0t, in_=w0r[:, :, half * (G // 2) : (half + 1) * (G // 2)]
                        )
                        w0h.append(w0t)
                    for tb in range(T // NT):
                        t0 = tb * NT
                        ew = win_pool.tile([128, K0C, NT], bf16, tag="ew", name=f"ew_{m}{d}_{tb}")
                        nc.sync.dma_start(out=ew, in_=embr[:, :, t0 : t0 + NT])
                        if d == 0:
                            mv = ew[:, :, :]
                        else:
                            # bwd dir: reversed time; psum col j = bwd-step
                            # s = (T - t0 - NT) + j
                            mv = ew[:, :, NT - 1 :: -1]
                        s0 = t0 if d == 0 else T - t0 - NT
                        for mc in range(GC):
                            wt = w0h[mc // 8]
                            mo = (mc % 8) * 128
                            ps = psx_pool.tile([128, NT], f32, tag="psx", name=f"ps0_{m}{d}_{tb}_{mc}")
                            for kc in range(K0C):
                                nc.tensor.matmul(
                                    ps, wt[:, kc, mo : mo + 128], mv[:, kc, :],
                                    start=(kc == 0), stop=(kc == K0C - 1),
                                )
                            st = xst_pool.tile([128, NT], bf16, tag="xst", name=f"st0_{m}{d}_{tb}_{mc}")
                            nc.vector.tensor_scalar(
                                out=st, in0=ps, scalar1=bias[:, mc : mc + 1],
                                scalar2=None, op0=ALU.add,
                            )
                            nc.sync.dma_start(
                                out=xp_dram[(m, 0, d)][:, mc * T + s0 : mc * T + s0 + NT],
                                in_=st,
                            )

            # ================= scan machinery ================================
            def pair_scan(m, l, Hs_f, Hs_b, init_h=None, init_c=None):
                """Interleaved fwd/bwd scan for stage (m, l). Hs_* are
                [128, HC*(T+1)] bf16 history tiles. init_h/init_c are
                ([128,16] bf16, [128,16] f32) tiles; columns 4*scan_idx.. hold
                the state for scan (l, d). Returns (c_f, c_b) f32 tiles."""
                dirs = []
                for d in (0, 1):
                    W = wslab_pool.tile([128, HC * G], bf16, tag="wslab", name=f"whh_{m}{l}_{d}")
                    nc.sync.dma_start(out=W, in_=whh_d[(m, l, d)][:, :])
                    hp = pw.tile([128, U + 1, HC], bf16, tag=f"hp{d}", name=f"hp_{m}{l}_{d}")
                    cd = pw.tile([128, HC], f32, tag=f"c{d}", name=f"c_{m}{l}_{d}")
                    si = 2 * l + d
                    if init_h is None:
                        nc.vector.memset(hp[:, 0, :], 0.0)
                        nc.vector.memset(cd, 0.0)
                    else:
                        nc.vector.tensor_copy(hp[:, 0, :], init_h[:, 4 * si : 4 * si + 4])
                        nc.vector.tensor_copy(cd, init_c[:, 4 * si : 4 * si + 4])
                    sg = pw.tile([128, 12], f32, tag=f"sg{d}", name=f"sg_{m}{l}_{d}")
                    tg = pw.tile([128, 4], f32, tag=f"tg{d}", name=f"tg_{m}{l}_{d}")
                    t1 = pw.tile([128, 4], f32, tag=f"t1{d}", name=f"t1_{m}{l}_{d}")
                    t2 = pw.tile([128, 4], f32, tag=f"t2{d}", name=f"t2_{m}{l}_{d}")
                    tn = pw.tile([128, 4], f32, tag=f"tn{d}", name=f"tn_{m}{l}_{d}")
                    dirs.append([W, hp, cd, sg, tg, t1, t2, tn, None])

                xpr = {
                    d: xp_dram[(m, l, d)][:, :].rearrange("p (g t) -> p g t", g=GC)
                    for d in (0, 1)
                }
                for w in range(T // CH):
                    t0 = w * CH
                    for d in (0, 1):
                        xwt = xw_pool.tile(
                            [128, GC, CH], bf16, tag=f"xw{d}", name=f"xw_{m}{l}_{d}_{w}"
                        )
                        nc.sync.dma_start(out=xwt, in_=xpr[d][:, :, t0 : t0 + CH])
                        dirs[d][8] = xwt
                    xc = [
                        pw.tile([128, GC, U], bf16, tag=f"xc{d}", name=f"xc_{m}{l}_{d}_{w}")
                        for d in (0, 1)
                    ]
                    with tc.For_i(0, CH // U) as iv:
                        for d in (0, 1):
                            nc.vector.tensor_copy(
                                xc[d],
                                dirs[d][8][:, :, ds(U * iv, U)],
                            )
                        for u in range(U):
                            pps = []
                            for d in (0, 1):
                                W, hp = dirs[d][0], dirs[d][1]
                                ps = pss_pool.tile(
                                    [128, GC], f32, tag=f"ps{d}", name=f"ps_{m}{l}_{d}_{u}"
                                )
                                nc.tensor.matmul(
                                    ps, ident, xc[d][:, :, u], start=True, stop=False,
                                    skip_group_check=True,
                                )
                                for mc in range(GC):
                                    for kc in range(HC):
                                        nc.tensor.matmul(
                                            ps[:, mc : mc + 1],
                                            W[:, kc * G + mc * 128 : kc * G + (mc + 1) * 128],
                                            hp[:, u, kc : kc + 1],
                                            start=False,
                                            stop=(kc == HC - 1),
                                            skip_group_check=True,
                                        )
                                pps.append(ps)
                            for d in (0, 1):
                                sg, tg = dirs[d][3], dirs[d][4]
                                nc.scalar.activation(sg, pps[d][:, 0:12], AF.Sigmoid)
                                nc.scalar.activation(tg, pps[d][:, 12:16], AF.Tanh)
                            for d in (0, 1):
                                _, _, cd, sg, tg, t1, t2, tn, _ = dirs[d]
                                nc.vector.tensor_tensor(out=t1, in0=sg[:, 4:8], in1=cd, op=ALU.mult)
                                nc.vector.tensor_tensor(out=t2, in0=sg[:, 0:4], in1=tg, op=ALU.mult)
                                nc.vector.tensor_tensor(out=cd, in0=t1, in1=t2, op=ALU.add)
                            for d in (0, 1):
                                nc.scalar.activation(dirs[d][7], dirs[d][2], AF.Tanh)
                            for d in (0, 1):
                                _, hp, _, sg, _, _, _, tn, _ = dirs[d]
                                nc.vector.tensor_tensor(
                                    out=hp[:, u + 1, :], in0=sg[:, 8:12], in1=tn, op=ALU.mult
                                )
                        # end of U steps: batch history copy + slot wrap
                        for d, tgt in ((0, Hs_f), (1, Hs_b)):
                            hp = dirs[d][1]
                            if tgt is not None:
                                nc.vector.tensor_copy(
                                    tgt[:, ds(HC * t0 + HC * U * iv + HC, HC * U)],
                                    hp[:, 1 : U + 1, :].rearrange("p u c -> p (u c)"),
                                )
                            nc.vector.tensor_copy(hp[:, 0, :], hp[:, U, :])
                return dirs[0][2], dirs[1][2], dirs[0][1], dirs[1][1]

            # L1 xp from L0 history tiles (both dirs local)
            def xp_l1(m, Hs_f, Hs_b):
                hfr = Hs_f[:, :].rearrange("p (t c) -> p t c", c=HC)
                hbr = Hs_b[:, :].rearrange("p (t c) -> p t c", c=HC)
                for d in (0, 1):
                    bias = pw.tile([128, GC], f32, tag="bias0", name=f"b1_{m}{d}")
                    nc.sync.dma_start(out=bias, in_=bias_d[(m, 1, d)][:, :])
                    wf = wslab_pool.tile([128, HC * G], bf16, tag="wslab", name=f"w1f_{m}{d}")
                    nc.sync.dma_start(out=wf, in_=wih1_d[(m, d, "f")][:, :])
                    wb = wslab_pool.tile([128, HC * G], bf16, tag="wslab", name=f"w1b_{m}{d}")
                    nc.sync.dma_start(out=wb, in_=wih1_d[(m, d, "b")][:, :])
                    for tb in range(T // NT):
                        s0 = tb * NT  # own-time index of the stored xp block
                        for mc in range(GC):
                            ps = psx_pool.tile([128, NT], f32, tag="psx", name=f"ps1_{m}{d}_{tb}_{mc}")
                            for kc in range(HC):
                                if d == 0:
                                    mv = hfr[:, s0 + 1 : s0 + 1 + NT, kc]
                                else:
                                    mv = hbr[:, s0 + 1 : s0 + 1 + NT, kc]
                                nc.tensor.matmul(
                                    ps,
                                    (wf if d == 0 else wb)[:, kc * G + mc * 128 : kc * G + (mc + 1) * 128],
                                    mv,
                                    start=(kc == 0), stop=False,
                                )
                            for kc in range(HC):
                                # other direction, read in own-time order:
                                # own step s <-> other-storage col T - s
                                if d == 0:
                                    mv = hbr[:, T - s0 : T - s0 - NT : -1, kc]
                                else:
                                    mv = hfr[:, T - s0 : T - s0 - NT : -1, kc]
                                nc.tensor.matmul(
                                    ps,
                                    (wb if d == 0 else wf)[:, kc * G + mc * 128 : kc * G + (mc + 1) * 128],
                                    mv,
                                    start=False, stop=(kc == HC - 1),
                                )
                            st = xst_pool.tile([128, NT], bf16, tag="xst", name=f"st1_{m}{d}_{tb}_{mc}")
                            nc.vector.tensor_scalar(
                                out=st, in0=ps, scalar1=bias[:, mc : mc + 1],
                                scalar2=None, op0=ALU.add,
                            )
                            nc.sync.dma_start(
                                out=xp_dram[(m, 1, d)][:, mc * T + s0 : mc * T + s0 + NT],
                                in_=st,
                            )

            # ================= ENC =========================================
            Hs_e0f = hs_pool.tile([128, HC * (T + 1)], bf16, tag="hs", name="Hs_e0f")
            Hs_e0b = hs_pool.tile([128, HC * (T + 1)], bf16, tag="hs", name="Hs_e0b")
            nc.vector.memset(Hs_e0f[:, 0:4], 0.0)
            nc.vector.memset(Hs_e0b[:, 0:4], 0.0)
            if "scan0" not in skips:
                c_e0f, c_e0b, hp_e0f, hp_e0b = pair_scan("enc", 0, Hs_e0f, Hs_e0b)
            else:
                c_e0f = pw.tile([128, HC], f32, name="dc0f"); nc.vector.memset(c_e0f, 0.0)
                c_e0b = pw.tile([128, HC], f32, name="dc0b"); nc.vector.memset(c_e0b, 0.0)
            if "xp1" not in skips:
                xp_l1("enc", Hs_e0f, Hs_e0b)
            if "scan1" not in skips:
                c_e1f, c_e1b, hp_e1f, hp_e1b = pair_scan("enc", 1, None, None)
            else:
                c_e1f, c_e1b = c_e0f, c_e0b
                hp_e1f = hp_e1b = None

            # ================= finals -> dec init states ====================
            # flat order [l0f l0b l1f l1b]
            fin_h = pw.tile([128, GC], bf16, name="fin_h")
            fin_c = pw.tile([128, GC], bf16, name="fin_c")
            for j, (hsrc, ct) in enumerate(
                (
                    (Hs_e0f[:, HC * T : HC * T + 4], c_e0f),
                    (Hs_e0b[:, HC * T : HC * T + 4], c_e0b),
                    (hp_e1f[:, 0, :] if hp_e1f is not None else Hs_e0f[:, 0:4], c_e1f),
                    (hp_e1b[:, 0, :] if hp_e1b is not None else Hs_e0b[:, 0:4], c_e1b),
                )
            ):
                nc.vector.tensor_copy(fin_h[:, 4 * j : 4 * j + 4], hsrc)
                nc.vector.tensor_copy(fin_c[:, 4 * j : 4 * j + 4], ct)

            init_h = pw.tile([128, GC], f32, name="init_h")
            init_c = pw.tile([128, GC], f32, name="init_c")
            for (wd, bd, fin, out_t) in (
                (e2hT_d, e2hb_d, fin_h, init_h),
                (e2cT_d, e2cb_d, fin_c, init_c),
            ):
                eb = pw.tile([128, GC], f32, tag="e2b", name=f"eb_{out_t.name}")
                nc.sync.dma_start(out=eb, in_=bd[:, :])
                wr = wd[:, :].rearrange("p (k g) -> p k g", k=GC)
                ps = psx_pool.tile([128, GC], f32, tag="psx", name=f"pse_{out_t.name}")
                for mc in range(GC):
                    eww = win_pool.tile([128, GC, 128], bf16, tag="ww", name=f"eww_{out_t.name}_{mc}")
                    nc.sync.dma_start(out=eww, in_=wr[:, :, mc * 128 : (mc + 1) * 128])
                    for kc in range(GC):
                        nc.tensor.matmul(
                            ps[:, mc : mc + 1],
                            eww[:, kc, :],
                            fin[:, kc : kc + 1],
                            start=(kc == 0), stop=(kc == GC - 1),
                        )
                nc.vector.tensor_tensor(out=out_t, in0=ps, in1=eb, op=ALU.add)
            init_h_bf = pw.tile([128, GC], bf16, name="init_h_bf")
            nc.vector.tensor_copy(init_h_bf, init_h)

            # ================= DEC =========================================
            Hs_d0f = hs_pool.tile([128, HC * (T + 1)], bf16, tag="hs", name="Hs_d0f")
            Hs_d0b = hs_pool.tile([128, HC * (T + 1)], bf16, tag="hs", name="Hs_d0b")
            nc.vector.memset(Hs_d0f[:, 0:4], 0.0)
            nc.vector.memset(Hs_d0b[:, 0:4], 0.0)
            if "scan2" not in skips:
                c_d0f, c_d0b, _, _ = pair_scan("dec", 0, Hs_d0f, Hs_d0b, init_h_bf, init_c)
            if "xp3" not in skips:
                xp_l1("dec", Hs_d0f, Hs_d0b)
            Hs_d1f = hs_pool.tile([128, HC * (T + 1)], bf16, tag="hs", name="Hs_d1f")
            Hs_d1b = hs_pool.tile([128, HC * (T + 1)], bf16, tag="hs", name="Hs_d1b")
            nc.vector.memset(Hs_d1f[:, 0:4], 0.0)
            nc.vector.memset(Hs_d1b[:, 0:4], 0.0)
            if "scan3" not in skips:
                c_d1f, c_d1b, _, _ = pair_scan("dec", 1, Hs_d1f, Hs_d1b, init_h_bf, init_c)

            # ================= feats =======================================
            h2tf = pw.tile([128, HC * K], bf16, name="h2tf")
            nc.sync.dma_start(out=h2tf, in_=h2tT_f_d[:, :])
            h2tb = pw.tile([128, HC * K], bf16, name="h2tb")
            nc.sync.dma_start(out=h2tb, in_=h2tT_b_d[:, :])
            h2tbias = pw.tile([K, 1], f32, name="h2tbias")
            nc.sync.dma_start(out=h2tbias, in_=h2tb_d[:, :])
            feats = pw.tile([K, T], f32, name="feats")
            d1fr = Hs_d1f[:, :].rearrange("p (t c) -> p t c", c=HC)
            d1br = Hs_d1b[:, :].rearrange("p (t c) -> p t c", c=HC)
            for tb in range(T // NT):
                t0 = tb * NT
                ps = psx_pool.tile([K, NT], f32, tag="psx", name=f"psf_{tb}")
                for kc in range(HC):
                    nc.tensor.matmul(
                        ps, h2tf[:, kc * K : (kc + 1) * K],
                        d1fr[:, t0 + 1 : t0 + 1 + NT, kc],
                        start=(kc == 0), stop=False,
                    )
                for kc in range(HC):
                    nc.tensor.matmul(
                        ps, h2tb[:, kc * K : (kc + 1) * K],
                        d1br[:, T - t0 : T - t0 - NT : -1, kc],
                        start=False, stop=(kc == HC - 1),
                    )
                nc.vector.tensor_scalar(
                    out=feats[:, t0 : t0 + NT], in0=ps, scalar1=h2tbias,
                    scalar2=None, op0=ALU.add,
                )
            nc.sync.dma_start(out=feats_out[:, :], in_=feats)

            # ================= CRF =========================================
            expF = feats
            nc.scalar.activation(expF, feats, AF.Exp)
            EexpT = pw.tile([K, K], f32, name="EexpT")
            nc.sync.dma_start(out=EexpT, in_=EexpT_d[:, :])
            Eexp = pw.tile([K, K], f32, name="Eexp")
            nc.sync.dma_start(out=Eexp, in_=Eexp_d[:, :])
            ones48 = pw.tile([K, K], f32, name="ones48")
            nc.sync.dma_start(out=ones48, in_=ones48_d[:, :])
            alpha = pw.tile([K, 1], f32, name="alpha")
            nc.sync.dma_start(out=alpha, in_=alpha0_d[:, :])
            beta = pw.tile([K, 1], f32, name="beta")
            nc.sync.dma_start(out=beta, in_=betaT_d[:, :])
            gam = pw.tile([K, 1], f32, name="gam")
            rsA = pw.tile([K, 1], f32, name="rsA")
            rsB = pw.tile([K, 1], f32, name="rsB")
            Ssb = pw.tile([1, NSA + NSB + 1], f32, name="Ssb")

            # alpha over t = 0..TH-1 ; beta over t = T-1..TH (gamma scaling).
            # beta tile holds beta_t; step i: gam = e_{T-1-i} * beta;
            # beta' = E^T @ gam. After TH steps beta = beta_{TH-1}.
            # ACT scalar operands cannot take register offsets -> prefetch the
            # body's expF columns into fixed tiles each iteration.
            ecA = pw.tile([K, RN], f32, name="ecA")
            ecB = pw.tile([K, RN], f32, name="ecB")
            with (tc.For_i(0, TH // RN) if "crf" not in skips else tc.For_i(0, 1)) as iv:
                nc.vector.tensor_copy(ecA, expF[:, ds(RN * iv, RN)])
                nc.vector.tensor_copy(ecB, expF[:, ds(T - RN - RN * iv, RN)])
                for u in range(RN):
                    # alpha: psA = E @ alpha ; alpha = e_t * psA
                    psA = pss_pool.tile([K, 1], f32, tag="ps0", name=f"psA_{u}")
                    nc.tensor.matmul(psA, EexpT, alpha, start=True, stop=True)
                    # beta: gam = e_{T-1-i} * beta (ACT), then psB = E^T @ gam
                    nc.scalar.activation(
                        gam, beta, AF.Copy, scale=ecB[:, RN - 1 - u : RN - u],
                    )
                    nc.scalar.activation(
                        alpha, psA, AF.Copy, scale=ecA[:, u : u + 1]
                    )
                    psB = pss_pool.tile([K, 1], f32, tag="ps1", name=f"psB_{u}")
                    nc.tensor.matmul(psB, Eexp, gam, start=True, stop=True)
                    nc.vector.tensor_copy(beta, psB)
                # renorm both streams; store norms
                psSA = pss_pool.tile([K, 1], f32, tag="ps0", name="psSA")
                nc.tensor.matmul(psSA, ones48, alpha, start=True, stop=True)
                nc.vector.reciprocal(rsA, psSA)
                nc.vector.tensor_tensor(out=alpha, in0=alpha, in1=rsA, op=ALU.mult)
                nc.vector.tensor_copy(Ssb[:, ds(iv, 1)], psSA[0:1, :])
                psSB = pss_pool.tile([K, 1], f32, tag="ps1", name="psSB")
                nc.tensor.matmul(psSB, ones48, beta, start=True, stop=True)
                nc.vector.reciprocal(rsB, psSB)
                nc.vector.tensor_tensor(out=beta, in0=beta, in1=rsB, op=ALU.mult)
                nc.vector.tensor_copy(Ssb[:, ds(NSA + iv, 1)], psSB[0:1, :])
            # final: dot(alpha_{TH-1}, beta_{TH-1})
            psZ = pss_pool.tile([1, 1], f32, tag="ps0", name="psZ")
            nc.tensor.matmul(psZ, alpha, beta, start=True, stop=True)
            nc.vector.tensor_copy(Ssb[:, NSA + NSB : NSA + NSB + 1], psZ)
            nc.sync.dma_start(out=snorm_out[:, :], in_=Ssb)
    nc.compile()
    return nc


# ----------------------------------------------------------------------------
# entry point
# ----------------------------------------------------------------------------

def _postprocess(r0, inputs):
    feats = r0["feats"].astype(np.float64)  # [K, T]
    sn = r0["snorm"].astype(np.float64)[0]
    Z = np.log(sn).sum()

    tags = np.asarray(inputs["tags"]).astype(np.int64)
    trans = np.asarray(inputs["transitions"]).astype(np.float64)
    ext = np.concatenate([[START_IDX], tags])
    score = trans[ext[1:], ext[:-1]].sum() + feats[tags, np.arange(T)].sum()
    score += trans[END_IDX, tags[-1]]
    return np.float32(Z - score)


def kernel(**inputs) -> np.ndarray:
    if "nc" not in _CACHE:
        _CACHE["nc"] = build()
    nc = _CACHE["nc"]
    in_map = _prep(inputs)
    res = run_bass_kernel_spmd(nc, [in_map], [0])
    return _postprocess(res.results[0], inputs)
